# revision 3
# baseline (speedup 1.0000x reference)
"""nn_MemoryAttention TP8 Trainium2 kernel v2.

fp16-max design: fp16 weights/activations everywhere the numpy precision
study allows, f32r for the wm projection (om converted per k-tile) and the
att@V path. Collective payloads fp16. W1/W3 streamed per chunk as n-major
slabs, W2 as quarters; WM resident f32r. x-side(t+1) overlaps AG1/AR,
yo(t-1) overlaps AG3. rstd2 folded into mem-scores pre-softmax and
mem-att weights post-softmax (om3 never materializes).
"""
import numpy as np

DIM = 2048; NH = 16; HD = 128; MEM = 128; SEQ = 2048; B = 4; HID = 5632
EPS = 1e-5
NC = 8
HPC = NH // NC          # 2 heads per core
FPC = DIM // NC         # 256
HIDP = 768              # padded per-core hidden
NCH = SEQ // MEM        # 16 chunks
TOK = B * MEM           # 512 token columns
KT = DIM // 128         # 16 feature k-tiles
HKT = HIDP // 128       # 6 hidden k-tiles

_RUNTIME = {}
COLL = True


def _trunc22(x):
    u = np.ascontiguousarray(x, np.float32).view(np.uint32)
    return ((u + np.uint32(0x200)) & np.uint32(0xFFFFFC00)).view(np.float32)


def _f16(x):
    return np.ascontiguousarray(np.asarray(x, np.float32).astype(np.float16))


def _head_perm():
    p = np.concatenate([np.arange(0, HD, 2), np.arange(1, HD, 2)])
    return np.concatenate([h * HD + p for h in range(NH)])


def _slab2(w):
    """[2048, 256] -> [128, KT, 2, 128] (p, k, n, f)."""
    return np.ascontiguousarray(
        w.reshape(KT, 128, 2, 128).transpose(1, 0, 2, 3))


def _slabn_nmajor(w, n):
    """[2048, n*128] -> [128, n, KT, 128] (p, n, k, f)."""
    return np.ascontiguousarray(
        w.reshape(KT, 128, n, 128).transpose(1, 2, 0, 3))


def _slab_w2_fmajor(w2):
    """[768, 2048] -> [128, 16, HKT, 128] (p, f, kk, j)."""
    return np.ascontiguousarray(
        w2.reshape(HKT, 128, 16, 128).transpose(1, 2, 0, 3))


def _kslab(w):
    """[2048, 256] -> [128, KT, 256] (moving-operand layout)."""
    return np.ascontiguousarray(w.reshape(KT, 128, FPC).transpose(1, 0, 2))


def _prepare(inputs):
    perm = _head_perm()
    scale = 1.0 / np.sqrt(HD)
    fw = np.asarray(inputs["ffn_norm_w"], np.float32)
    mw = np.asarray(inputs["mem_norm_w"], np.float32)
    wq = np.asarray(inputs["wq"])[:, perm] * scale
    wk = np.asarray(inputs["wk"])[:, perm]
    wkm = (mw[:, None] * np.asarray(inputs["wkm"]))[:, perm]
    wv = np.asarray(inputs["wv"])
    wvm = mw[:, None] * np.asarray(inputs["wvm"])
    wm = np.asarray(inputs["wm"]); wo = np.asarray(inputs["wo"])
    w1 = np.zeros((DIM, NC * HIDP), np.float32)
    w3 = np.zeros((DIM, NC * HIDP), np.float32)
    w2 = np.zeros((NC * HIDP, DIM), np.float32)
    w1s = fw[:, None] * np.asarray(inputs["w1"])
    w3s = fw[:, None] * np.asarray(inputs["w3"])
    for c in range(NC):
        w1[:, c * HIDP:c * HIDP + 704] = w1s[:, c * 704:(c + 1) * 704]
        w3[:, c * HIDP:c * HIDP + 704] = w3s[:, c * 704:(c + 1) * 704]
        w2[c * HIDP:c * HIDP + 704] = np.asarray(inputs["w2"])[c * 704:(c + 1) * 704]
    fc = np.asarray(inputs["freqs_cos"]); fs = np.asarray(inputs["freqs_sin"])
    cs_mem = _trunc22(np.concatenate(
        [np.tile(fc[0:MEM].T, (1, B)), np.tile(fs[0:MEM].T, (1, B))], 0))
    cs_x = _trunc22(np.concatenate(
        [np.tile(fc[MEM:2 * MEM].T, (1, B)),
         np.tile(fs[MEM:2 * MEM].T, (1, B))], 0))
    mask = np.zeros((MEM, MEM), np.float32)
    for i in range(MEM):
        mask[i, i + 1:] = -1e30
    x = np.asarray(inputs["x"])
    xT = _f16(x.reshape(B, NCH, MEM, DIM).transpose(1, 3, 0, 2)
              .reshape(NCH, DIM, TOK))
    om0 = np.asarray(inputs["origin_mem"])
    omT0 = _f16(om0.transpose(2, 0, 1).reshape(DIM, TOK))
    in_maps = []
    for c in range(NC):
        hsl = slice(c * FPC, (c + 1) * FPC)
        hidsl = slice(c * HIDP, (c + 1) * HIDP)
        in_maps.append({
            "WQ": _f16(_slab2(wq[:, hsl])),
            "WK": _f16(_slab2(wk[:, hsl])),
            "WKM": _f16(_slab2(wkm[:, hsl])),
            "WM": _trunc22(_slab2(wm[:, hsl])),
            "WO": _f16(_slab2(wo[:, hsl])),
            "W1": _f16(_slabn_nmajor(w1[:, hidsl], HKT)),
            "W3": _f16(_slabn_nmajor(w3[:, hidsl], HKT)),
            "W2": _f16(_slab_w2_fmajor(w2[hidsl, :])),
            "WV": _f16(_kslab(wv[:, hsl])),
            "WVM": _f16(_kslab(wvm[:, hsl])),
            "XT": xT, "OM0T": omT0,
            "CSM": cs_mem, "CSX": cs_x,
            "MASK": mask,
        })
    return in_maps


def _build():
    import concourse.bacc as bacc
    import concourse.tile as tile
    import concourse.mybir as mybir
    from concourse.masks import make_identity
    from contextlib import ExitStack

    dt = mybir.dt
    AluOp = mybir.AluOpType
    AFT = mybir.ActivationFunctionType
    f32, f32r, f16 = dt.float32, dt.float32r, dt.float16

    nc = bacc.Bacc("TRN2", target_bir_lowering=False, debug=False,
                   num_devices=NC)

    def din(name, shape, dtype):
        return nc.dram_tensor(name, shape, dtype, kind="ExternalInput")

    WQ = din("WQ", [128, KT, 2, 128], f16)
    WK = din("WK", [128, KT, 2, 128], f16)
    WKM = din("WKM", [128, KT, 2, 128], f16)
    WM = din("WM", [128, KT, 2, 128], f32r)
    WO = din("WO", [128, KT, 2, 128], f16)
    W1 = din("W1", [128, HKT, KT, 128], f16)
    W3 = din("W3", [128, HKT, KT, 128], f16)
    W2 = din("W2", [128, 16, HKT, 128], f16)
    WV = din("WV", [128, KT, FPC], f16)
    WVM = din("WVM", [128, KT, FPC], f16)
    XT = din("XT", [NCH, DIM, TOK], f16)
    OM0T = din("OM0T", [DIM, TOK], f16)
    CSM = din("CSM", [128, TOK], f32)
    CSX = din("CSX", [128, TOK], f32)
    MASK = din("MASK", [MEM, MEM], f32)
    YO = nc.dram_tensor("YO", [NCH, FPC, TOK], f16, kind="ExternalOutput")

    rg = [list(range(NC))]
    SH = "Shared" if COLL else "Local"

    with tile.TileContext(nc) as tc:
        es = ExitStack()
        const = es.enter_context(tc.tile_pool(name="const", bufs=1))
        wres = es.enter_context(tc.tile_pool(name="wres", bufs=1))
        wslab = es.enter_context(tc.tile_pool(name="wslab", bufs=2))
        w13p = es.enter_context(tc.tile_pool(name="w13p", bufs=2))
        w2p = es.enter_context(tc.tile_pool(name="w2p", bufs=2))
        wvp = es.enter_context(tc.tile_pool(name="wvp", bufs=1))
        omp = es.enter_context(tc.tile_pool(name="omp", bufs=1))
        omcv = es.enter_context(tc.tile_pool(name="omcv", bufs=2))
        gpool = es.enter_context(tc.tile_pool(name="gpool", bufs=1))
        xpool = es.enter_context(tc.tile_pool(name="xpool", bufs=1))
        qkpool = es.enter_context(tc.tile_pool(name="qkpool", bufs=2))
        kmpool = es.enter_context(tc.tile_pool(name="kmpool", bufs=1))
        vpool = es.enter_context(tc.tile_pool(name="vpool", bufs=2))
        stg = es.enter_context(tc.tile_pool(name="stg", bufs=2))
        attp = es.enter_context(tc.tile_pool(name="attp", bufs=2))
        smol = es.enter_context(tc.tile_pool(name="smol", bufs=1))
        bcp = es.enter_context(tc.tile_pool(name="bcp", bufs=1))
        dram = es.enter_context(tc.tile_pool(name="dram", bufs=1, space="DRAM"))
        psA = es.enter_context(tc.tile_pool(name="psA", bufs=3, space="PSUM"))
        psV = es.enter_context(tc.tile_pool(name="psV", bufs=1, space="PSUM"))
        psS = es.enter_context(tc.tile_pool(name="psS", bufs=1, space="PSUM"))
        psT = es.enter_context(tc.tile_pool(name="psT", bufs=1, space="PSUM"))

        # ---- constants
        csm = const.tile([128, TOK], f32); nc.sync.dma_start(csm[:], CSM[:])
        csx = const.tile([128, TOK], f32); nc.sync.dma_start(csx[:], CSX[:])
        maskt = const.tile([MEM, MEM], f32)
        nc.sync.dma_start(maskt[:], MASK[:])
        scr32 = const.tile([128, 128], f32)
        nc.vector.memset(scr32[:], 1.0)
        ones = const.tile([128, 1], f32r)
        nc.vector.tensor_copy(ones[:], scr32[:, 0:1])
        onesrow = const.tile([1, 128], f32r)
        nc.vector.tensor_copy(onesrow[:], scr32[0:1, :])
        make_identity(nc, scr32)
        ident = const.tile([128, 128], f16)
        nc.vector.tensor_copy(ident[:], scr32[:])
        epst = const.tile([1, 1], f32)
        nc.vector.memset(epst[:], EPS)
        # resident f32r wm
        wms = wres.tile([128, KT, 2, 128], f32r, name="wms")
        nc.sync.dma_start(wms[:], WM[:])

        om16 = omp.tile([128, KT, TOK], f16, tag="om", name="om_init", bufs=1)
        nc.sync.dma_start(om16[:], OM0T[:].rearrange("(k p) t -> p k t", p=128))

        def mm(p, lhsT, rhs, start, stop):
            nc.tensor.matmul(p, lhsT, rhs, start=start, stop=stop)

        def rope2(dst, src01, cs):
            """src01: two [128, TOK] psum views; dst [128, 2, TOK] f16."""
            cosT, sinT = cs[0:64, :], cs[64:128, :]
            for h in range(2):
                ph = src01[h]
                r, i = ph[0:64, :], ph[64:128, :]
                t1 = stg.tile([64, TOK], f32, tag="t1", name="t1")
                t2 = stg.tile([64, TOK], f32, tag="t2", name="t2")
                nc.vector.tensor_mul(t1[:], r, cosT)
                nc.vector.tensor_mul(t2[:], i, sinT)
                nc.vector.tensor_sub(dst[0:64, h, :], t1[:], t2[:])
                t3 = stg.tile([64, TOK], f32, tag="t1", name="t3")
                t4 = stg.tile([64, TOK], f32, tag="t2", name="t4")
                nc.vector.tensor_mul(t3[:], r, sinT)
                nc.vector.tensor_mul(t4[:], i, cosT)
                nc.vector.tensor_add(dst[64:128, h, :], t3[:], t4[:])

        def rstd_chain(ssq, tag):
            """ssq psum [1,TOK] -> bc sbuf [128,TOK] f32."""
            rstd = smol.tile([1, TOK], f32, tag="rstd", name="rstd", bufs=2)
            nc.scalar.activation(rstd[:], ssq[:], AFT.Sqrt,
                                 bias=epst[:], scale=1.0 / DIM)
            rec_r = smol.tile([1, TOK], f32r, tag="recr", name="recr", bufs=2)
            with nc.allow_low_precision(reason="f32r rstd broadcast by design"):
                nc.vector.reciprocal(rec_r[:], rstd[:])
            bcps = psA.tile([128, TOK], f32, tag="mm", name="bcps")
            mm(bcps[:], onesrow[:], rec_r[:], True, True)
            bc = bcp.tile([128, TOK], f32, tag=f"bc{tag}", name="bc")
            nc.vector.tensor_copy(bc[:], bcps[:])
            return bc

        def x_side_q(t):
            """x chunk DMA + xq proj + rope."""
            xk = xpool.tile([128, KT, TOK], f16, tag="xk", name="xk")
            nc.sync.dma_start(xk[:],
                              XT[t].rearrange("(k p) t2 -> p k t2", p=128))
            ws = wslab.tile([128, KT, 2, 128], f16, tag="wslab", name="wqs")
            nc.sync.dma_start(ws[:], WQ[:])
            ps = [psA.tile([128, TOK], f32, tag="mm", name=f"pq{n}")
                  for n in range(2)]
            for k in range(KT):
                for n in range(2):
                    mm(ps[n][:], ws[:, k, n, :], xk[:, k, :], k == 0, k == KT - 1)
            qT = qkpool.tile([128, 2, TOK], f16, tag="qT", name="qT")
            rope2(qT, [p[:] for p in ps], csx)
            return xk, qT

        def x_side_k(xk):
            ws = wslab.tile([128, KT, 2, 128], f16, tag="wslab", name="wks")
            nc.sync.dma_start(ws[:], WK[:])
            ps = [psA.tile([128, TOK], f32, tag="mm", name=f"pk{n}")
                  for n in range(2)]
            for k in range(KT):
                for n in range(2):
                    mm(ps[n][:], ws[:, k, n, :], xk[:, k, :], k == 0, k == KT - 1)
            kxT = qkpool.tile([128, 2, TOK], f16, tag="kxT", name="kxT")
            rope2(kxT, [p[:] for p in ps], csx)
            return kxT

        def x_side_v(xk):
            wv = wvp.tile([128, KT, FPC], f16, tag="wv", name="wv")
            nc.sync.dma_start(wv[:], WV[:])
            vx = vpool.tile([128, B, FPC], f16, tag="vx", name="vx")
            for b in range(B):
                pv = psV.tile([128, FPC], f32, tag="pv", name="pv")
                for k in range(KT):
                    mm(pv[:], xk[:, k, b * 128:(b + 1) * 128], wv[:, k, :],
                       k == 0, k == KT - 1)
                nc.vector.tensor_copy(vx[:, b, :], pv[:])
            return vx

        def allgather(in_d, out_d):
            if COLL:
                nc.gpsimd.collective_compute(
                    "AllGather", AluOp.bypass, replica_groups=rg,
                    ins=[in_d[:].opt()], outs=[out_d[:].opt()])
            else:
                n = out_d.shape[0] // in_d.shape[0]
                sz = in_d.shape[0]
                for r in range(n):
                    nc.sync.dma_start(out_d[r * sz:(r + 1) * sz, :], in_d[:])

        def allreduce(in_d, out_d):
            if COLL:
                nc.gpsimd.collective_compute(
                    "AllReduce", AluOp.add, replica_groups=rg,
                    ins=[in_d[:].opt()], outs=[out_d[:].opt()])
            else:
                nc.sync.dma_start(out_d[:], in_d[:])

        def yo_proj(om_t, t_out):
            ws = wslab.tile([128, KT, 2, 128], f16, tag="wslab", name="wos")
            nc.sync.dma_start(ws[:], WO[:])
            for n in range(2):
                p = psA.tile([128, TOK], f32, tag="mm", name="pyo")
                for k in range(KT):
                    mm(p[:], ws[:, k, n, :], om_t[:, k, :], k == 0, k == KT - 1)
                o = stg.tile([128, TOK], f16, tag="io", name="yo", bufs=1)
                nc.vector.tensor_copy(o[:], p[:])
                nc.sync.dma_start(YO[t_out, n * 128:(n + 1) * 128, :], o[:])

        xk0, qT = x_side_q(0)
        kxT = x_side_k(xk0)
        vx = x_side_v(xk0)

        for t in range(NCH):
            # ---- x-side q for t+1 (fills om16-reload gap)
            if t + 1 < NCH:
                xk_n, qT_n = x_side_q(t + 1)
            # ---- wm proj (om f16 -> f32r convert per k-tile) + AG1
            pw = [psA.tile([128, TOK], f32, tag="mm", name=f"pw{n}")
                  for n in range(2)]
            for k in range(KT):
                oc = omcv.tile([128, TOK], f32r, tag="oc", name="oc")
                nc.gpsimd.tensor_copy(oc[:], om16[:, k, :])
                for n in range(2):
                    mm(pw[n][:], wms[:, k, n, :], oc[:], k == 0, k == KT - 1)
            ag1s = stg.tile([128, 2, TOK], f16, tag="ag1", name="ag1s", bufs=1)
            for n in range(2):
                nc.vector.tensor_copy(ag1s[:, n, :], pw[n][:])
            agin1 = dram.tile([FPC, TOK], f16, name=f"agin1_{t}")
            for n in range(2):
                nc.sync.dma_start(agin1[n * 128:(n + 1) * 128, :],
                                  ag1s[:, n, :])
            agout1 = dram.tile([DIM, TOK], f16, addr_space=SH,
                               name=f"agout1_{t}")
            allgather(agin1, agout1)
            # ---- overlap AG1: xk(t+1)
            if t + 1 < NCH:
                kxT_n = x_side_k(xk_n)
            # ---- om1 + norm1
            om1 = omp.tile([128, KT, TOK], f16, tag="om1", name="om1")
            nc.sync.dma_start(om1[:],
                              agout1[:].rearrange("(k p) t2 -> p k t2", p=128))
            ssq = psS.tile([1, TOK], f32, tag="ssq", name="ssq1")
            for k in range(KT):
                sq = stg.tile([128, TOK], f32r, tag="sq", name="sq")
                nc.gpsimd.tensor_mul(sq[:], om1[:, k, :], om1[:, k, :])
                mm(ssq[:], ones[:], sq[:], k == 0, k == KT - 1)
            bc1 = rstd_chain(ssq, "1")
            # ---- FFN up (W1/W3 streamed per n-slab)
            g = gpool.tile([128, HKT, TOK], f16, tag="g", name="g")
            for n in range(HKT):
                w1n = w13p.tile([128, KT, 128], f16, tag="w13", name="w1n")
                nc.sync.dma_start(w1n[:], W1[:, n, :, :])
                w3n = w13p.tile([128, KT, 128], f16, tag="w13", name="w3n")
                nc.sync.dma_start(w3n[:], W3[:, n, :, :])
                p1 = psA.tile([128, TOK], f32, tag="mm", name="p1")
                p3 = psA.tile([128, TOK], f32, tag="mm", name="p3")
                for k in range(KT):
                    mm(p1[:], w1n[:, k, :], om1[:, k, :], k == 0, k == KT - 1)
                for k in range(KT):
                    mm(p3[:], w3n[:, k, :], om1[:, k, :], k == 0, k == KT - 1)
                t1 = stg.tile([128, TOK], f32, tag="ff", name="t1")
                nc.vector.tensor_mul(t1[:], p1[:], bc1[:])
                sil = stg.tile([128, TOK], f32, tag="ff2", name="sil")
                nc.scalar.activation(sil[:], t1[:], AFT.Silu)
                t3 = stg.tile([128, TOK], f32, tag="ff", name="t3")
                nc.vector.tensor_mul(t3[:], p3[:], bc1[:])
                nc.vector.tensor_mul(g[:, n, :], sil[:], t3[:])
            # ---- FFN down + residual + split AR (W2 streamed per quarter)
            arouts = []
            for half in range(2):
                arin = dram.tile([DIM // 2, TOK], f16, name=f"arin_{t}_{half}")
                for quarter in range(2):
                    w2q = w2p.tile([128, 4, HKT, 128], f16, tag="w2q",
                                   name="w2q")
                    fbase = half * 8 + quarter * 4
                    nc.sync.dma_start(w2q[:], W2[:, fbase:fbase + 4, :, :])
                    for i in range(4):
                        nf = fbase + i
                        p = psA.tile([128, TOK], f32, tag="mm", name="pd")
                        for kk in range(HKT):
                            mm(p[:], w2q[:, i, kk, :], g[:, kk, :],
                               kk == 0, kk == HKT - 1)
                        o = stg.tile([128, TOK], f16, tag="ar", name="fo")
                        nc.vector.scalar_tensor_tensor(
                            o[:], om1[:, nf, :], 1.0 / NC, p[:],
                            op0=AluOp.mult, op1=AluOp.add)
                        nc.sync.dma_start(
                            arin[(nf - half * 8) * 128:(nf - half * 8 + 1) * 128, :],
                            o[:])
                arout = dram.tile([DIM // 2, TOK], f16, addr_space=SH,
                                  name=f"arout_{t}_{half}")
                allreduce(arin, arout)
                arouts.append(arout)
            # ---- overlap AR: xv(t+1)
            if t + 1 < NCH:
                vx_n = x_side_v(xk_n)
            # ---- om2 + norm2 + km (separate half tiles: per-tile deps
            #      let half-0 work genuinely overlap AR half-1)
            om2h = [omp.tile([128, 8, TOK], f16, tag=f"om2{hh}",
                             name=f"om2{hh}") for hh in range(2)]

            def om2k(k):
                return om2h[k // 8][:, k % 8, :]

            wkms = wslab.tile([128, KT, 2, 128], f16, tag="wslab", name="wkms")
            nc.sync.dma_start(wkms[:], WKM[:])
            ssq2 = psS.tile([1, TOK], f32, tag="ssq", name="ssq2")
            pk = [psA.tile([128, TOK], f32, tag="mm", name=f"pkm{n}")
                  for n in range(2)]
            for half in range(2):
                nc.sync.dma_start(
                    om2h[half][:],
                    arouts[half][:].rearrange("(k p) t2 -> p k t2", p=128))
                for k in range(half * 8, half * 8 + 8):
                    sq = stg.tile([128, TOK], f32r, tag="sq", name="sq2")
                    nc.gpsimd.tensor_mul(sq[:], om2k(k), om2k(k))
                    mm(ssq2[:], ones[:], sq[:], k == 0, k == KT - 1)
                for k in range(half * 8, half * 8 + 8):
                    for n in range(2):
                        mm(pk[n][:], wkms[:, k, n, :], om2k(k),
                           k == 0, k == KT - 1)
            bc2 = rstd_chain(ssq2, "2")
            kmT = kmpool.tile([128, 2, TOK], f16, tag="kmT", name="kmT")
            rope2(kmT, [p[:] for p in pk], csm)
            # ---- vm projection (stationary om2 slabs, moving WVM)
            wvm = wvp.tile([128, KT, FPC], f16, tag="wvm", name="wvm")
            nc.sync.dma_start(wvm[:], WVM[:])
            vm = vpool.tile([128, B, FPC], f16, tag="vm", name="vm", bufs=1)
            for b in range(B):
                pv = psV.tile([128, FPC], f32, tag="pv", name="pvm")
                for k in range(KT):
                    mm(pv[:], om2k(k)[:, b * 128:(b + 1) * 128],
                       wvm[:, k, :], k == 0, k == KT - 1)
                nc.vector.tensor_copy(vm[:, b, :], pv[:])
            # ---- attention (chunk queries only; rstd2 folded into
            #      mem-scores pre-softmax and mem-att weights post-softmax)
            aout = stg.tile([128, 2, TOK], f16, tag="aout", name="aout",
                            bufs=1)
            for h_ in range(HPC):
                for b in range(B):
                    bs = slice(b * 128, (b + 1) * 128)
                    ps = psT.tile([128, 2 * MEM], f32, tag="ps", name="ps")
                    mm(ps[:, 0:128], qT[:, h_, bs], kmT[:, h_, bs], True, True)
                    mm(ps[:, 128:256], qT[:, h_, bs], kxT[:, h_, bs], True, True)
                    s = attp.tile([128, 2 * MEM], f32, tag="s", name="s", bufs=1)
                    nc.vector.tensor_mul(s[:, 0:128], ps[:, 0:128], bc2[:, bs])
                    nc.vector.tensor_add(s[:, 128:256], ps[:, 128:256],
                                         maskt[:])
                    negmax = smol.tile([128, 1], f32, tag="negmax", name="nm",
                                       bufs=4)
                    nc.vector.tensor_reduce(negmax[:], s[:],
                                            mybir.AxisListType.X, AluOp.max,
                                            negate=True)
                    e = attp.tile([128, 2 * MEM], f32, tag="e", name="e")
                    den = smol.tile([128, 1], f32, tag="den", name="den",
                                    bufs=4)
                    nc.scalar.activation(e[:], s[:], AFT.Exp,
                                         bias=negmax[:], scale=1.0,
                                         accum_out=den[:])
                    rec = smol.tile([128, 1], f32, tag="arec", name="arec",
                                    bufs=4)
                    nc.vector.reciprocal(rec[:], den[:])
                    att = attp.tile([128, 2 * MEM], f16, tag="att",
                                    name="att", bufs=1)
                    nc.vector.scalar_tensor_tensor(
                        att[:, 0:128], e[:, 0:128], rec[:], bc2[:, bs],
                        op0=AluOp.mult, op1=AluOp.mult)
                    nc.vector.tensor_scalar_mul(att[:, 128:256],
                                                e[:, 128:256], rec[:])
                    attT = attp.tile([128, 2, 128], f16, tag="attT",
                                     name="attT", bufs=1)
                    for half in range(2):
                        pt = psT.tile([128, 128], f16, tag="pt", name="pt")
                        nc.tensor.transpose(
                            pt[:], att[:, half * 128:(half + 1) * 128],
                            ident[:])
                        nc.vector.tensor_copy(attT[:, half, :], pt[:])
                    po = psT.tile([128, 128], f32, tag="po", name="po")
                    mm(po[:], vm[:, b, h_ * 128:(h_ + 1) * 128],
                       attT[:, 0, :], True, False)
                    mm(po[:], vx[:, b, h_ * 128:(h_ + 1) * 128],
                       attT[:, 1, :], False, True)
                    nc.vector.tensor_copy(aout[:, h_, bs], po[:])
            # ---- AG3 -> next om; yo(t-1) overlaps
            agin3 = dram.tile([FPC, TOK], f16, name=f"agin3_{t}")
            for h_ in range(HPC):
                nc.sync.dma_start(agin3[h_ * 128:(h_ + 1) * 128, :],
                                  aout[:, h_, :])
            agout3 = dram.tile([DIM, TOK], f16, addr_space=SH,
                               name=f"agout3_{t}")
            allgather(agin3, agout3)
            if t > 0:
                yo_proj(om16, t - 1)
            om16 = omp.tile([128, KT, TOK], f16, tag="om", name=f"om_{t + 1}",
                            bufs=1)
            nc.sync.dma_start(om16[:],
                              agout3[:].rearrange("(k p) t2 -> p k t2", p=128))
            if t + 1 < NCH:
                xk0, qT, kxT, vx = xk_n, qT_n, kxT_n, vx_n
        yo_proj(om16, NCH - 1)
        es.close()

    nc.compile()
    return nc


def _get_runtime():
    if "nc" not in _RUNTIME:
        _RUNTIME["nc"] = _build()
    return _RUNTIME["nc"]


def _assemble(results):
    out = np.zeros((B, SEQ, DIM), np.float32)
    for c in range(NC):
        yo = np.asarray(results[c]["YO"], np.float32)  # [NCH, FPC, TOK]
        y = yo.reshape(NCH, FPC, B, MEM).transpose(2, 0, 3, 1)
        out[:, :, c * FPC:(c + 1) * FPC] = y.reshape(B, SEQ, FPC)
    return out


def kernel(**inputs):
    from concourse.bass_utils import run_bass_kernel_spmd
    nc = _get_runtime()
    in_maps = _prepare(inputs)
    res = run_bass_kernel_spmd(nc, in_maps, core_ids=list(range(NC)),
                               trace=False)
    return _assemble(res.results)


if __name__ == "__main__":
    _build()
    print("build ok")
